# revision 1
# baseline (speedup 1.0000x reference)
"""Trainium2 Bass kernel: batched single-head attention + gate MLP.

Per-core (data-parallel over batch, 1 batch row per core):
  q = query @ Wq.T + bq ; k,v likewise
  scores = q @ k.T / sqrt(768); attn = softmax(scores)
  attended = attn @ v
  h = relu(attended @ Wg1.T + bg1); gate = sigmoid(h @ Wg2.T + bg2)
  out = sigmoid(gate) * attended * text_scale

Matmuls run in float32r (TF32-like, 12-bit mantissa) at full PE rate.
All contractions put the reduced dim on partitions, so the three input
tensors and the five weights are transposed on the PE via identity
matmuls. qT is bounced through DRAM to fit SBUF.
"""
import numpy as np

import concourse.bass as bass
import concourse.mybir as mybir
import concourse.tile as tile
from concourse import bacc
from concourse.bass_utils import run_bass_kernel_spmd

F32 = mybir.dt.float32
F32R = mybir.dt.float32r
AF = mybir.ActivationFunctionType

B, S, D = 8, 2048, 768
EB = D // 128           # 6 blocks of the feature dim
SB = S // 128           # 16 blocks of the seq dim
PCH = 512               # projection s-chunk
NPCH = S // PCH         # 4
ICH = 256               # attention/gate i-chunk
NICH = S // ICH         # 8
SCALE = 1.0 / float(np.sqrt(D))

_CACHE = {}


def _build(reps=1):
    nc = bacc.Bacc(None)

    query = nc.dram_tensor("query", [S, D], F32, kind="ExternalInput")
    key = nc.dram_tensor("key", [S, D], F32, kind="ExternalInput")
    value = nc.dram_tensor("value", [S, D], F32, kind="ExternalInput")
    Wq = nc.dram_tensor("Wq", [D, D], F32, kind="ExternalInput")
    Wk = nc.dram_tensor("Wk", [D, D], F32, kind="ExternalInput")
    Wv = nc.dram_tensor("Wv", [D, D], F32, kind="ExternalInput")
    Wg1 = nc.dram_tensor("Wg1", [D, D], F32, kind="ExternalInput")
    Wg2 = nc.dram_tensor("Wg2", [D, D], F32, kind="ExternalInput")
    bq = nc.dram_tensor("bq", [D], F32, kind="ExternalInput")
    bk = nc.dram_tensor("bk", [D], F32, kind="ExternalInput")
    bv = nc.dram_tensor("bv", [D], F32, kind="ExternalInput")
    bg1 = nc.dram_tensor("bg1", [D], F32, kind="ExternalInput")
    bg2 = nc.dram_tensor("bg2", [D], F32, kind="ExternalInput")
    ts = nc.dram_tensor("ts", [1, D], F32, kind="ExternalInput")
    ident = nc.dram_tensor("ident", [128, 128], F32, kind="ExternalInput")
    ones = nc.dram_tensor("ones", [128, 128], F32, kind="ExternalInput")
    out = nc.dram_tensor("out", [S, D], F32, kind="ExternalOutput")

    with tile.TileContext(nc) as tc:
        with tc.tile_pool(name="persist", bufs=1) as P, \
             tc.tile_pool(name="psc", bufs=2, space="PSUM") as PSC, \
             tc.tile_pool(name="pmm", bufs=2, space="PSUM") as PMM, \
             tc.tile_pool(name="dram", bufs=1, space="DRAM") as DR:

            ident_sb = P.tile([128, 128], F32R, tag="ident")
            nc.gpsimd.dma_start(out=ident_sb, in_=ident[:, :])
            ones_sb = P.tile([128, 128], F32R, tag="ones")
            nc.gpsimd.dma_start(out=ones_sb, in_=ones[:, :])

            kT = P.tile([128, EB, S], F32R, tag="kT")        # k^T [e, s]
            v_sb = P.tile([128, SB, D], F32R, tag="v")       # v [j, e]

            def vec_sb(name, src):                           # [D] -> [128, EB]
                t = P.tile([128, EB], F32, tag=name)
                nc.sync.dma_start(out=t, in_=src.rearrange("(b p) -> p b", p=128))
                return t

            bq_sb = vec_sb("bq", bq[:])
            bk_sb = vec_sb("bk", bk[:])
            bg1_sb = vec_sb("bg1", bg1[:])
            bg2_sb = vec_sb("bg2", bg2[:])
            ts_sb = vec_sb("ts", ts[0, :])
            bg2h_sb = P.tile([128, EB], F32, tag="bg2h")
            nc.vector.tensor_scalar_mul(bg2h_sb, bg2_sb, 0.5)
            tsh_sb = P.tile([128, EB], F32, tag="tsh")
            nc.vector.tensor_scalar_mul(tsh_sb, ts_sb, 0.5)

            qT_dram = DR.tile([D, S], F32R, tag="qTdram")

            def load_wT(wdram, wT, pool):
                """DMA W [e,d] fp32, transpose on PE, cast to f32r on evict."""
                wst = pool.tile([128, EB, D], F32R, tag="wstage", bufs=1)
                nc.gpsimd.dma_start(
                    out=wst, in_=wdram.rearrange("(eb p) d -> p eb d", p=128))
                for db in range(EB):
                    for eb0 in range(0, EB, 3):
                        tp = PSC.tile([128, 384], F32R, tag="sc")
                        for k in range(3):
                            nc.tensor.transpose(
                                tp[:, k * 128:(k + 1) * 128],
                                wst[:, eb0 + k, db * 128:(db + 1) * 128], ident_sb)
                        nc.vector.tensor_copy(
                            wT[:, db, eb0 * 128:(eb0 + 3) * 128], tp)

            def load_xT(xdram, c, pool, tag):
                """DMA input s-chunk c (cast->f32r) + transpose -> [p, db, s]."""
                nsb = PCH // 128
                xst = pool.tile([128, nsb, D], F32R, tag=tag + "st", bufs=2)
                nc.gpsimd.dma_start(
                    out=xst,
                    in_=xdram[c * PCH:(c + 1) * PCH, :].rearrange(
                        "(sb p) d -> p sb d", p=128))
                xT = pool.tile([128, EB, PCH], F32R, tag=tag + "T", bufs=1)
                for sb in range(nsb):
                    for db0 in range(0, EB, 3):
                        tp = PSC.tile([128, 3, 128], F32R, tag="sc")
                        for k in range(3):
                            nc.tensor.transpose(
                                tp[:, k, :],
                                xst[:, sb, (db0 + k) * 128:(db0 + k + 1) * 128],
                                ident_sb)
                        nc.vector.tensor_copy(
                            xT[:, db0:db0 + 3, sb * 128:(sb + 1) * 128], tp)
                return xT

            for _rep in range(reps):
                # ---- Phase A: project key -> kT, value -> v ----
                with tc.tile_pool(name="phA", bufs=2) as PA:
                    wkT = PA.tile([128, EB, D], F32R, tag="wkT", bufs=1)
                    bv_bc = PA.tile([128, D], F32, tag="bv", bufs=1)
                    nc.sync.dma_start(out=bv_bc, in_=bv[:].partition_broadcast(128))
                    wvT = PA.tile([128, EB, D], F32R, tag="wvT", bufs=1)
                    load_wT(Wk, wkT, PA)
                    load_wT(Wv, wvT, PA)
                    for c in range(NPCH):
                        kxT = load_xT(key, c, PA, "x")
                        for eb in range(EB):
                            ps = PSC.tile([128, PCH], F32, tag="sc")
                            for db in range(EB):
                                nc.tensor.matmul(
                                    ps, wkT[:, db, eb * 128:(eb + 1) * 128],
                                    kxT[:, db, :], start=(db == 0), stop=(db == EB - 1))
                            nc.scalar.activation(
                                kT[:, eb, c * PCH:(c + 1) * PCH], ps, AF.Identity,
                                bias=bk_sb[:, eb:eb + 1])
                        vxT = load_xT(value, c, PA, "x")
                        for jb in range(PCH // 128):
                            pv = PMM.tile([128, D], F32, tag="mm")
                            for n0, n1 in ((0, 512), (512, 768)):
                                for db in range(EB):
                                    nc.tensor.matmul(
                                        pv[:, n0:n1],
                                        vxT[:, db, jb * 128:(jb + 1) * 128],
                                        wvT[:, db, n0:n1],
                                        start=(db == 0), stop=(db == EB - 1))
                            nc.vector.tensor_add(
                                v_sb[:, c * (PCH // 128) + jb, :], pv[:, 0:D], bv_bc)

                # ---- Phase B: project query -> qT (DRAM bounce); load gate W ----
                persist2 = tc.tile_pool(name="persist2", bufs=1)
                P2 = persist2.__enter__()
                wg1T = P2.tile([128, EB, D], F32R, tag="wg1T")
                wg2T = P2.tile([128, EB, D], F32R, tag="wg2T")
                with tc.tile_pool(name="phB", bufs=2) as PB:
                    wqT = PB.tile([128, EB, D], F32R, tag="wqT", bufs=1)
                    load_wT(Wq, wqT, PB)
                    load_wT(Wg1, wg1T, PB)
                    load_wT(Wg2, wg2T, PB)
                    for c in range(NPCH):
                        qxT = load_xT(query, c, PB, "x")
                        for eb in range(EB):
                            ps = PSC.tile([128, PCH], F32, tag="sc")
                            for db in range(EB):
                                nc.tensor.matmul(
                                    ps, wqT[:, db, eb * 128:(eb + 1) * 128],
                                    qxT[:, db, :], start=(db == 0), stop=(db == EB - 1))
                            qrow = PB.tile([128, PCH], F32R, tag="qrow", bufs=1)
                            nc.scalar.activation(
                                qrow, ps, AF.Identity, bias=bq_sb[:, eb:eb + 1])
                            nc.sync.dma_start(
                                out=qT_dram[eb * 128:(eb + 1) * 128,
                                            c * PCH:(c + 1) * PCH],
                                in_=qrow)

                # ---- Phase C: attention + gate, i-chunks of ICH ----
                with tc.tile_pool(name="phC", bufs=2) as PC, \
                     tc.tile_pool(name="phC1", bufs=1) as PC1:
                    nib = ICH // 128
                    for ic in range(NICH):
                        qTc = PC.tile([128, EB, ICH], F32R, tag="qTc", bufs=1)
                        nc.sync.dma_start(
                            out=qTc,
                            in_=qT_dram[:, ic * ICH:(ic + 1) * ICH].rearrange(
                                "(eb p) i -> p eb i", p=128))
                        attnT = PC1.tile([128, SB, ICH], F32R, tag="attnT")
                        for jb in range(SB):
                            ps = PSC.tile([128, ICH], F32, tag="sc")
                            for eb in range(EB):
                                nc.tensor.matmul(
                                    ps, kT[:, eb, jb * 128:(jb + 1) * 128],
                                    qTc[:, eb, :],
                                    start=(eb == 0), stop=(eb == EB - 1))
                            nc.scalar.activation(
                                attnT[:, jb, :], ps, AF.Exp, scale=SCALE)
                        # denominator, replicated on all partitions: ones^T @ exp
                        sps = PSC.tile([128, ICH], F32, tag="sc")
                        for jb in range(SB):
                            nc.tensor.matmul(
                                sps, ones_sb, attnT[:, jb, :],
                                start=(jb == 0), stop=(jb == SB - 1))
                        recip_bc = PC1.tile([128, ICH], F32, tag="recipbc")
                        nc.vector.reciprocal(recip_bc, sps)
                        # attendedT [e_blk, i]
                        pa = PMM.tile([128, EB, ICH], F32, tag="mm")
                        for eb in range(EB):
                            for jb in range(SB):
                                nc.tensor.matmul(
                                    pa[:, eb, :], v_sb[:, jb, eb * 128:(eb + 1) * 128],
                                    attnT[:, jb, :], start=(jb == 0), stop=(jb == SB - 1))
                        attT = PC.tile([128, EB, ICH], F32R, tag="attT", bufs=1)
                        for eb in range(EB):
                            nc.vector.tensor_mul(
                                attT[:, eb, :], pa[:, eb, :], recip_bc)
                        # hT = relu(Wg1 @ attended + bg1)
                        ph = PMM.tile([128, EB, ICH], F32, tag="mm")
                        for e2 in range(EB):
                            for eb in range(EB):
                                nc.tensor.matmul(
                                    ph[:, e2, :], wg1T[:, eb, e2 * 128:(e2 + 1) * 128],
                                    attT[:, eb, :], start=(eb == 0), stop=(eb == EB - 1))
                        hT = PC.tile([128, EB, ICH], F32R, tag="hT", bufs=1)
                        for e2 in range(EB):
                            nc.scalar.activation(
                                hT[:, e2, :], ph[:, e2, :], AF.Relu,
                                bias=bg1_sb[:, e2:e2 + 1])
                        # gateT = sigmoid(Wg2 @ h + bg2); then sigmoid again
                        pg = PMM.tile([128, EB, ICH], F32, tag="mm")
                        for e2 in range(EB):
                            for eb in range(EB):
                                nc.tensor.matmul(
                                    pg[:, e2, :], wg2T[:, eb, e2 * 128:(e2 + 1) * 128],
                                    hT[:, eb, :], start=(eb == 0), stop=(eb == EB - 1))
                        g2 = PC.tile([128, EB, ICH], F32, tag="g2", bufs=1)
                        for e2 in range(EB):
                            nc.scalar.activation(
                                g2[:, e2, :], pg[:, e2, :], AF.Tanh,
                                bias=bg2h_sb[:, e2:e2 + 1], scale=0.5)
                        nc.vector.tensor_scalar(
                            g2, g2, 0.5, 0.5, mybir.AluOpType.mult,
                            mybir.AluOpType.add)
                        g3 = PC.tile([128, EB, ICH], F32, tag="g3", bufs=1)
                        nc.scalar.activation(g3, g2, AF.Tanh, scale=0.5)
                        av = PC.tile([128, EB, ICH], F32, tag="av", bufs=1)
                        for eb in range(EB):
                            nc.vector.tensor_scalar_mul(
                                av[:, eb, :], attT[:, eb, :], tsh_sb[:, eb:eb + 1])
                        gated = PC.tile([128, EB, ICH], F32R, tag="gated", bufs=1)
                        nc.vector.tensor_mul(gated, g3, av)
                        nc.vector.tensor_add(gated, gated, av)

                        # transpose back to [s, e] and store
                        for ib in range(nib):
                            po = PMM.tile([128, D], F32R, tag="mm")
                            for eb in range(EB):
                                nc.tensor.transpose(
                                    po[:, eb * 128:(eb + 1) * 128],
                                    gated[:, eb, ib * 128:(ib + 1) * 128], ident_sb)
                            osb = PC.tile([128, D], F32, tag="osb", bufs=1)
                            nc.vector.tensor_copy(osb, po)
                            r0 = (ic * nib + ib) * 128
                            nc.sync.dma_start(out=out[r0:r0 + 128, :], in_=osb)

                persist2.__exit__(None, None, None)

    nc.compile()
    return nc


def kernel(**inputs):
    if "nc" not in _CACHE:
        _CACHE["nc"] = _build()
    nc = _CACHE["nc"]
    inputs = dict(inputs)
    q = np.ascontiguousarray(inputs["query"], dtype=np.float32)
    k = np.ascontiguousarray(inputs["key"], dtype=np.float32)
    vv = np.ascontiguousarray(inputs["value"], dtype=np.float32)
    shared = {
        "Wq": np.ascontiguousarray(inputs["Wq"], np.float32),
        "Wk": np.ascontiguousarray(inputs["Wk"], np.float32),
        "Wv": np.ascontiguousarray(inputs["Wv"], np.float32),
        "Wg1": np.ascontiguousarray(inputs["Wg1"], np.float32),
        "Wg2": np.ascontiguousarray(inputs["Wg2"], np.float32),
        "bq": np.ascontiguousarray(inputs["bq"], np.float32),
        "bk": np.ascontiguousarray(inputs["bk"], np.float32),
        "bv": np.ascontiguousarray(inputs["bv"], np.float32),
        "bg1": np.ascontiguousarray(inputs["bg1"], np.float32),
        "bg2": np.ascontiguousarray(inputs["bg2"], np.float32),
        "ts": np.ascontiguousarray(inputs["text_scale"], np.float32),
        "ident": np.eye(128, dtype=np.float32),
        "ones": np.ones((128, 128), dtype=np.float32),
    }
    in_maps = [
        dict(shared, query=q[b], key=k[b], value=vv[b]) for b in range(B)
    ]
    trace = bool(inputs.get("_trace"))
    r = run_bass_kernel_spmd(nc, in_maps, list(range(B)), trace=trace)
    if trace:
        print("HW exec time:", r.exec_time_ns, "ns")
        _CACHE["last_result"] = r
    return np.stack([r.results[b]["out"] for b in range(B)], axis=0)


if __name__ == "__main__":
    rng = np.random.default_rng(0)
    pass

